# revision 28
# baseline (speedup 1.0000x reference)
"""Trainium2 Bass kernel for causal multi-head attention with RoPE.

Problem: x[2,2048,2048], 16 heads, head_dim 128, fp32.
  q/k/v = x @ w{q,k,v}^T ; RoPE on q,k ; causal softmax(q k^T / sqrt(128)) @ v ; out @ wo^T

Sharding: Megatron tensor-parallel over heads — 2 heads per core on 8 cores.
Each core computes a partial y (its 2 heads' contribution through wo); the host
sums the 8 partials.  No device collectives.

Per-core layout strategy (all matmuls fp32r at free-dim >= 256, probs bf16):
  - xT [2048, 4096]  (feature-major activations, host-pre-transposed)
  - q^T, k^T computed feature-major [head_dim, tokens]; v token-major [tokens, d]
  - scores computed transposed: S^T[key, q] = kT.T @ qT  (single K=128 pass)
  - softmax WITHOUT max subtraction (scores bounded ~ +-10, exp is safe in fp32):
      P^T = exp(S^T / sqrt(128)) (ACT engine, fused scale), causal mask by
      multiplying bf16 0/1 mask tiles, row-sum r via ones-column matmul,
      o^T = v.T @ P^T accumulated in PSUM, normalized by broadcast(1/r).
  - y rows = (o_norm^T).T @ woT, written token-major straight to DRAM.
"""

import math
import sys

sys.path.insert(0, "/opt/trn_rl_repo")

import numpy as np  # noqa: E402

P = 128
D = 2048
HD = 128  # head dim
B = 2
T = 2048
TOK = B * T  # 4096
NCORES = 8
HPC = 2  # heads per core
DC = HPC * HD  # 256 dims per core
CCHUNKS = D // P  # 16 contraction chunks
TT = TOK // 512  # 8 token tiles of 512
QT = T // 512  # 4 query tiles per batch
KT_PER_Q = 512 // P  # 4 key tiles per query tile

_CACHE = {}


def _build_nc():
    import concourse.bacc as bacc
    import concourse.mybir as mybir
    import concourse.tile as tile

    f32 = mybir.dt.float32
    f32r = mybir.dt.float32r
    bf16 = mybir.dt.bfloat16

    nc = bacc.Bacc("TRN2", target_bir_lowering=False, debug=False, num_devices=NCORES)

    # x pre-tiled on host: [tt, c_chunk, 128, 512], each chunk contiguous
    xTt = nc.dram_tensor("xTt", [TT, CCHUNKS, P, 512], f32r,
                         kind="ExternalInput").ap()
    cosT = nc.dram_tensor("cosT", [HD, TOK], f32, kind="ExternalInput").ap()
    sinT = nc.dram_tensor("sinT", [HD, TOK], f32, kind="ExternalInput").ap()
    wqT = nc.dram_tensor("wqT", [D, DC], f32r, kind="ExternalInput").ap()
    wkT = nc.dram_tensor("wkT", [D, DC], f32r, kind="ExternalInput").ap()
    wvT = nc.dram_tensor("wvT", [D, DC], f32r, kind="ExternalInput").ap()
    woT = nc.dram_tensor("woT", [DC, D], f32r, kind="ExternalInput").ap()
    y = nc.dram_tensor("y", [TOK, D], f32, kind="ExternalOutput").ap()

    inv_sqrt_hd = 1.0 / math.sqrt(HD)

    with tile.TileContext(nc) as tc:
        with (
            tc.tile_pool(name="consts", bufs=1) as consts,
            tc.tile_pool(name="wpool", bufs=1) as wpool,
            tc.tile_pool(name="qkv", bufs=1) as qkv,
            tc.tile_pool(name="xp", bufs=3) as xp,
            tc.tile_pool(name="csp", bufs=2) as csp,
            tc.tile_pool(name="ropep", bufs=2) as ropep,
            tc.tile_pool(name="ptp", bufs=4) as ptp,
            tc.tile_pool(name="rrp", bufs=2) as rrp,
            tc.tile_pool(name="bcp", bufs=2) as bcp,
            tc.tile_pool(name="onp", bufs=3) as onp,
            tc.tile_pool(name="ysp", bufs=2) as ysp,
            tc.tile_pool(name="ps", bufs=8, space="PSUM") as ps,
        ):
            # ---- constants ----
            # causal 0/1 bf16 masks for the 4 diagonal-crossing offsets
            masks = []
            for mi in range(KT_PER_Q):
                m = consts.tile([P, 512], bf16, tag=f"mask{mi}")
                nc.gpsimd.memset(m[:], 1.0)
                # keep where (q_local - key_local) >= 0:  f - p - 128*mi >= 0
                nc.gpsimd.affine_select(
                    out=m[:], in_=m[:], compare_op=mybir.AluOpType.is_ge,
                    fill=0.0, base=-P * mi, channel_multiplier=-1, pattern=[[1, 512]],
                )
                masks.append(m)
            ones_col = consts.tile([P, 1], bf16, tag="ones_col")
            nc.gpsimd.memset(ones_col[:], 1.0)

            # ---- resident weights.  Per-c-chunk DMAs are emitted inside the
            # first token tile's c-loop so the x-tile stream is not queued
            # behind 8 MiB of weight traffic; wo loads after phase 1. ----
            wq_t = wpool.tile([P, CCHUNKS, DC], f32r, tag="wq")
            wk_t = wpool.tile([P, CCHUNKS, DC], f32r, tag="wk")
            wv_t = wpool.tile([P, CCHUNKS, DC], f32r, tag="wv")
            wo_t = wpool.tile([P, HPC, D], f32r, tag="wo")

            def emit_w_chunk(c):
                for wt, wdram in ((wq_t, wqT), (wk_t, wkT), (wv_t, wvT)):
                    nc.sync.dma_start(
                        wt[:, c:c + 1, :],
                        wdram.rearrange("(co ci) d -> ci co d", ci=P)[:, c:c + 1, :])

            # ---- resident activations ----
            qT_t = qkv.tile([P, HPC, TOK], f32r, tag="qT")  # [head_dim, h, tok]
            kT_t = qkv.tile([P, HPC, TOK], f32r, tag="kT")
            v_t = qkv.tile([P, TOK // P, DC], bf16, tag="v")  # [tok%128, tokblk, d]

            # ---- phase 1 tile body ----
            def emit_tile(tt):
                tsl = slice(tt * 512, (tt + 1) * 512)
                cos_t = csp.tile([P, 512], f32, tag="cos")
                nc.scalar.dma_start(cos_t[:], cosT[:, tsl])
                sin_t = csp.tile([P, 512], f32, tag="sin")
                nc.scalar.dma_start(sin_t[:], sinT[:, tsl])

                pq = [ps.tile([P, 512], f32, tag="ps", name=f"pq{i}") for i in range(HPC)]
                pk = [ps.tile([P, 512], f32, tag="ps", name=f"pk{i}") for i in range(HPC)]
                # two banks hold all four v accumulators ([t128, 256] pairs packed
                # side by side).  Only the first half's c==0 matmul uses start=True
                # (clears the whole bank); the second half's first matmul then
                # overwrites its still-clean elements via has_written bits.
                pv = [ps.tile([P, 512], f32, tag="ps", name=f"pv{i}") for i in range(2)]

                for c in range(CCHUNKS):
                    if tt == 0:
                        emit_w_chunk(c)
                    xt = xp.tile([P, 512], f32r, tag="x")
                    nc.sync.dma_start(xt[:], xTt[tt, c])
                    xtr = xt[:]
                    st, sp = (c == 0), (c == CCHUNKS - 1)
                    for h in range(HPC):
                        dsl = slice(h * HD, (h + 1) * HD)
                        nc.tensor.matmul(pq[h][:], wq_t[:, c, dsl], xtr,
                                         start=st, stop=sp)
                        nc.tensor.matmul(pk[h][:], wk_t[:, c, dsl], xtr,
                                         start=st, stop=sp)
                    for s4 in range(4):
                        half = s4 % 2
                        nc.tensor.matmul(pv[s4 // 2][:, half * DC:(half + 1) * DC],
                                         xt[:, s4 * P:(s4 + 1) * P],
                                         wv_t[:, c, :],
                                         start=st and half == 0, stop=sp,
                                         skip_group_check=half == 1)

                # Free all six PSUM banks as fast as possible: raw q + v copies
                # on ACT, raw k copies on DVE (parallel engines), then run RoPE
                # in place from SBUF.
                for h in range(HPC):
                    nc.scalar.copy(qT_t[:, h, tsl], pq[h][:])
                for h in range(HPC):
                    nc.vector.tensor_copy(kT_t[:, h, tsl], pk[h][:])
                for s4 in range(4):
                    half = s4 % 2
                    nc.scalar.copy(v_t[:, tt * 4 + s4, :],
                                   pv[s4 // 2][:, half * DC:(half + 1) * DC])
                # RoPE: dst = raw*cos + rot(raw)*sin (rot: [0:64]=-raw[64:], [64:]=raw[:64])
                for dst_t in (qT_t, kT_t):
                    for h in range(HPC):
                        dst = dst_t[:, h, tsl]
                        rot = ropep.tile([P, 512], f32, tag="rot")
                        nc.scalar.mul(rot[0:64, :], dst[64:128, :], -1.0)
                        nc.scalar.copy(rot[64:128, :], dst[0:64, :])
                        nc.vector.tensor_mul(out=rot[:], in0=rot[:], in1=sin_t[:])
                        nc.vector.tensor_mul(out=dst, in0=dst, in1=cos_t[:])
                        nc.vector.tensor_add(out=dst, in0=dst, in1=rot[:])

            # ---- phase 2: attention + output projection ----
            # yproj of unit i is emitted after attention of unit i+1 (software
            # pipelining): the PE then has scores/AV matmuls to run while unit
            # i's normalization chain (recip -> broadcast -> mul) completes.
            def emit_yproj(onorm, b, qt):
                for s4 in range(4):
                    r0 = b * T + qt * 512 + s4 * P
                    ystage = ysp.tile([P, D], f32, tag="ystage")
                    for dout in range(4):
                        py = ps.tile([P, 512], f32, tag="ps", name="py")
                        for h in range(HPC):
                            nc.tensor.matmul(
                                py[:],
                                onorm[:, h, s4 * P:(s4 + 1) * P],
                                wo_t[:, h, dout * 512:(dout + 1) * 512],
                                start=(h == 0), stop=(h == HPC - 1))
                        nc.scalar.copy(ystage[:, dout * 512:(dout + 1) * 512], py[:])
                    nc.sync.dma_start(y[r0:r0 + P, :], ystage[:])

            pending = []

            def emit_attn(b, qt):
                    qsl = slice(b * T + qt * 512, b * T + qt * 512 + 512)
                    onorm = onp.tile([P, HPC, 512], f32r, tag="onorm")
                    for h in range(HPC):
                        qr = qT_t[:, h, qsl]
                        nkt = KT_PER_Q * (qt + 1)
                        po = ps.tile([P, 512], f32, tag="ps")
                        pr = ps.tile([P, 512], f32, tag="ps")

                        def emit_score(kt, b=b, qt=qt, h=h, qr=qr):
                            ksl = slice(b * T + kt * P, b * T + (kt + 1) * P)
                            pscore = ps.tile([P, 512], f32, tag="ps", name="pscore")
                            nc.tensor.matmul(pscore[:], kT_t[:, h, ksl],
                                             qr, start=True, stop=True)
                            ptile = ptp.tile([P, 512], bf16, tag="pt", name="ptile")
                            nc.scalar.activation(ptile[:], pscore[:],
                                                 mybir.ActivationFunctionType.Exp,
                                                 scale=inv_sqrt_hd)
                            if kt >= KT_PER_Q * qt:
                                nc.vector.tensor_mul(out=ptile[:], in0=ptile[:],
                                                     in1=masks[kt - KT_PER_Q * qt][:])
                            return ptile

                        # kt loop pipelined by one: scores for kt+1 are issued
                        # before the exp-gated AV/ones matmuls of kt, so the PE
                        # always has wait-free work while ACT runs exp.
                        ptiles = {0: emit_score(0)}
                        for kt in range(nkt):
                            if kt + 1 < nkt:
                                ptiles[kt + 1] = emit_score(kt + 1)
                            ptile = ptiles.pop(kt)
                            st, sp = (kt == 0), (kt == nkt - 1)
                            # ones first: its 1-column weight load is free, and
                            # the AV matmul's 128-col weight load prefetches
                            # during the ones stream (no wait between them).
                            nc.tensor.matmul(pr[0:1, :], ones_col[:], ptile[:],
                                             start=st, stop=sp)
                            nc.tensor.matmul(po[:], v_t[:, b * (T // P) + kt,
                                                        h * HD:(h + 1) * HD],
                                             ptile[:], start=st, stop=sp)
                        # copy o out of PSUM right away (frees the bank), then
                        # normalize in place once 1/r is broadcast.
                        nc.scalar.copy(onorm[:, h, :], po[:])
                        rr = rrp.tile([1, 512], f32, tag="rr")
                        nc.vector.reciprocal(rr[:], pr[0:1, :])
                        bc = bcp.tile([P, 512], f32, tag="bc")
                        nc.gpsimd.partition_broadcast(bc[:], rr[:])
                        nc.vector.tensor_mul(out=onorm[:, h, :],
                                             in0=onorm[:, h, :], in1=bc[:])

                    pending.append((onorm, b, qt))
                    if len(pending) > 2:
                        emit_yproj(*pending.pop(0))

            # ---- schedule ----
            for tt in range(TT):
                emit_tile(tt)
                if tt == 3:
                    for h in range(HPC):
                        nc.scalar.dma_start(
                            wo_t[:, h, :],
                            woT.rearrange("(ko ki) n -> ki ko n", ki=P)[:, h, :])
            for b in range(B):
                for qt in range(QT):
                    emit_attn(b, qt)
            for p_ in pending:
                emit_yproj(*p_)

    nc.compile()
    return nc


def get_nc():
    if "nc" not in _CACHE:
        _CACHE["nc"] = _build_nc()
    return _CACHE["nc"]


def make_in_maps(x, cos, sin, wq, wk, wv, wo):
    xT = x.reshape(TOK, D).T  # [D, TOK]
    xTt = np.ascontiguousarray(
        xT.reshape(CCHUNKS, P, TT, 512).transpose(2, 0, 1, 3))
    cosT = np.ascontiguousarray(cos.reshape(TOK, HD).T)
    sinT = np.ascontiguousarray(sin.reshape(TOK, HD).T)
    in_maps = []
    for c in range(NCORES):
        dsl = slice(c * DC, (c + 1) * DC)
        in_maps.append({
            "xTt": xTt,
            "cosT": cosT,
            "sinT": sinT,
            "wqT": np.ascontiguousarray(wq[dsl, :].T),
            "wkT": np.ascontiguousarray(wk[dsl, :].T),
            "wvT": np.ascontiguousarray(wv[dsl, :].T),
            "woT": np.ascontiguousarray(wo[:, dsl].T),
        })
    return in_maps


def kernel(x, cos, sin, wq, wk, wv, wo):
    from concourse.bass_utils import run_bass_kernel_spmd

    nc = get_nc()
    in_maps = make_in_maps(
        np.asarray(x, dtype=np.float32), np.asarray(cos, dtype=np.float32),
        np.asarray(sin, dtype=np.float32), np.asarray(wq, dtype=np.float32),
        np.asarray(wk, dtype=np.float32), np.asarray(wv, dtype=np.float32),
        np.asarray(wo, dtype=np.float32))
    res = run_bass_kernel_spmd(nc, in_maps, list(range(NCORES)))
    out = np.zeros((TOK, D), dtype=np.float64)
    for m in res.results:
        out += m["y"].astype(np.float64)
    return out.astype(np.float32).reshape(B, T, D)


# revision 29
# speedup vs baseline: 1.0250x; 1.0250x over previous
"""Trainium2 Bass kernel for causal multi-head attention with RoPE.

Problem: x[2,2048,2048], 16 heads, head_dim 128, fp32.
  q/k/v = x @ w{q,k,v}^T ; RoPE on q,k ; causal softmax(q k^T / sqrt(128)) @ v ; out @ wo^T

Sharding: Megatron tensor-parallel over heads — 2 heads per core on 8 cores.
Each core computes a partial y (its 2 heads' contribution through wo); the host
sums the 8 partials.  No device collectives.

Per-core layout strategy (all matmuls fp32r at free-dim >= 256, probs bf16):
  - xT [2048, 4096]  (feature-major activations, host-pre-transposed)
  - q^T, k^T computed feature-major [head_dim, tokens]; v token-major [tokens, d]
  - scores computed transposed: S^T[key, q] = kT.T @ qT  (single K=128 pass)
  - softmax WITHOUT max subtraction (scores bounded ~ +-10, exp is safe in fp32):
      P^T = exp(S^T / sqrt(128)) (ACT engine, fused scale), causal mask by
      multiplying bf16 0/1 mask tiles, row-sum r via ones-column matmul,
      o^T = v.T @ P^T accumulated in PSUM, normalized by broadcast(1/r).
  - y rows = (o_norm^T).T @ woT, written token-major straight to DRAM.
"""

import math
import sys

sys.path.insert(0, "/opt/trn_rl_repo")

import numpy as np  # noqa: E402

P = 128
D = 2048
HD = 128  # head dim
B = 2
T = 2048
TOK = B * T  # 4096
NCORES = 8
HPC = 2  # heads per core
DC = HPC * HD  # 256 dims per core
CCHUNKS = D // P  # 16 contraction chunks
TT = TOK // 512  # 8 token tiles of 512
QT = T // 512  # 4 query tiles per batch
KT_PER_Q = 512 // P  # 4 key tiles per query tile

_CACHE = {}


def _build_nc():
    import concourse.bacc as bacc
    import concourse.mybir as mybir
    import concourse.tile as tile

    f32 = mybir.dt.float32
    f32r = mybir.dt.float32r
    bf16 = mybir.dt.bfloat16

    nc = bacc.Bacc("TRN2", target_bir_lowering=False, debug=False, num_devices=NCORES)

    # x pre-tiled on host: [tt, c_chunk, 128, 512], each chunk contiguous
    xTt = nc.dram_tensor("xTt", [TT, CCHUNKS, P, 512], f32r,
                         kind="ExternalInput").ap()
    cosT = nc.dram_tensor("cosT", [HD, TOK], f32, kind="ExternalInput").ap()
    sinT = nc.dram_tensor("sinT", [HD, TOK], f32, kind="ExternalInput").ap()
    wqT = nc.dram_tensor("wqT", [D, DC], f32r, kind="ExternalInput").ap()
    wkT = nc.dram_tensor("wkT", [D, DC], f32r, kind="ExternalInput").ap()
    wvT = nc.dram_tensor("wvT", [D, DC], f32r, kind="ExternalInput").ap()
    woT = nc.dram_tensor("woT", [DC, D], f32r, kind="ExternalInput").ap()
    y = nc.dram_tensor("y", [TOK, D], f32, kind="ExternalOutput").ap()

    inv_sqrt_hd = 1.0 / math.sqrt(HD)

    with tile.TileContext(nc) as tc:
        with (
            tc.tile_pool(name="consts", bufs=1) as consts,
            tc.tile_pool(name="wpool", bufs=1) as wpool,
            tc.tile_pool(name="qkv", bufs=1) as qkv,
            tc.tile_pool(name="xp", bufs=3) as xp,
            tc.tile_pool(name="csp", bufs=2) as csp,
            tc.tile_pool(name="ropep", bufs=2) as ropep,
            tc.tile_pool(name="ptp", bufs=4) as ptp,
            tc.tile_pool(name="rrp", bufs=2) as rrp,
            tc.tile_pool(name="bcp", bufs=2) as bcp,
            tc.tile_pool(name="onp", bufs=3) as onp,
            tc.tile_pool(name="ysp", bufs=2) as ysp,
            tc.tile_pool(name="ps", bufs=8, space="PSUM") as ps,
        ):
            # ---- constants ----
            # causal 0/1 bf16 masks for the 4 diagonal-crossing offsets
            masks = []
            for mi in range(KT_PER_Q):
                m = consts.tile([P, 512], bf16, tag=f"mask{mi}")
                nc.gpsimd.memset(m[:], 1.0)
                # keep where (q_local - key_local) >= 0:  f - p - 128*mi >= 0
                nc.gpsimd.affine_select(
                    out=m[:], in_=m[:], compare_op=mybir.AluOpType.is_ge,
                    fill=0.0, base=-P * mi, channel_multiplier=-1, pattern=[[1, 512]],
                )
                masks.append(m)
            ones_col = consts.tile([P, 1], bf16, tag="ones_col")
            nc.gpsimd.memset(ones_col[:], 1.0)

            # ---- resident weights.  Per-c-chunk DMAs are emitted inside the
            # first token tile's c-loop so the x-tile stream is not queued
            # behind 8 MiB of weight traffic; wo loads after phase 1. ----
            wq_t = wpool.tile([P, CCHUNKS, DC], f32r, tag="wq")
            wk_t = wpool.tile([P, CCHUNKS, DC], f32r, tag="wk")
            wv_t = wpool.tile([P, CCHUNKS, DC], f32r, tag="wv")
            wo_t = wpool.tile([P, HPC, D], f32r, tag="wo")

            def emit_w_chunk(c):
                for wt, wdram in ((wq_t, wqT), (wk_t, wkT), (wv_t, wvT)):
                    nc.sync.dma_start(
                        wt[:, c:c + 1, :],
                        wdram.rearrange("(co ci) d -> ci co d", ci=P)[:, c:c + 1, :])

            # ---- resident activations ----
            qT_t = qkv.tile([P, HPC, TOK], f32r, tag="qT")  # [head_dim, h, tok]
            kT_t = qkv.tile([P, HPC, TOK], f32r, tag="kT")
            v_t = qkv.tile([P, TOK // P, DC], bf16, tag="v")  # [tok%128, tokblk, d]

            # ---- phase 1 tile body ----
            def emit_tile(tt):
                tsl = slice(tt * 512, (tt + 1) * 512)
                cos_t = csp.tile([P, 512], f32, tag="cos")
                nc.scalar.dma_start(cos_t[:], cosT[:, tsl])
                sin_t = csp.tile([P, 512], f32, tag="sin")
                nc.scalar.dma_start(sin_t[:], sinT[:, tsl])

                pq = [ps.tile([P, 512], f32, tag="ps", name=f"pq{i}") for i in range(HPC)]
                pk = [ps.tile([P, 512], f32, tag="ps", name=f"pk{i}") for i in range(HPC)]
                # two banks hold all four v accumulators ([t128, 256] pairs packed
                # side by side).  Only the first half's c==0 matmul uses start=True
                # (clears the whole bank); the second half's first matmul then
                # overwrites its still-clean elements via has_written bits.
                pv = [ps.tile([P, 512], f32, tag="ps", name=f"pv{i}") for i in range(2)]

                for c in range(CCHUNKS):
                    if tt == 0:
                        emit_w_chunk(c)
                    xt = xp.tile([P, 512], f32r, tag="x")
                    nc.sync.dma_start(xt[:], xTt[tt, c])
                    xtr = xt[:]
                    st, sp = (c == 0), (c == CCHUNKS - 1)
                    for h in range(HPC):
                        dsl = slice(h * HD, (h + 1) * HD)
                        nc.tensor.matmul(pq[h][:], wq_t[:, c, dsl], xtr,
                                         start=st, stop=sp)
                        nc.tensor.matmul(pk[h][:], wk_t[:, c, dsl], xtr,
                                         start=st, stop=sp)
                    for s4 in range(4):
                        half = s4 % 2
                        nc.tensor.matmul(pv[s4 // 2][:, half * DC:(half + 1) * DC],
                                         xt[:, s4 * P:(s4 + 1) * P],
                                         wv_t[:, c, :],
                                         start=st and half == 0, stop=sp,
                                         skip_group_check=half == 1)

                # Free all six PSUM banks as fast as possible: raw q + v copies
                # on ACT, raw k copies on DVE (parallel engines), then run RoPE
                # in place from SBUF.
                for h in range(HPC):
                    nc.scalar.copy(qT_t[:, h, tsl], pq[h][:])
                for h in range(HPC):
                    nc.vector.tensor_copy(kT_t[:, h, tsl], pk[h][:])
                for s4 in range(4):
                    half = s4 % 2
                    nc.scalar.copy(v_t[:, tt * 4 + s4, :],
                                   pv[s4 // 2][:, half * DC:(half + 1) * DC])
                # RoPE: dst = raw*cos + rot(raw)*sin (rot: [0:64]=-raw[64:], [64:]=raw[:64])
                for dst_t in (qT_t, kT_t):
                    for h in range(HPC):
                        dst = dst_t[:, h, tsl]
                        rot = ropep.tile([P, 512], f32, tag="rot")
                        nc.scalar.mul(rot[0:64, :], dst[64:128, :], -1.0)
                        nc.scalar.copy(rot[64:128, :], dst[0:64, :])
                        nc.vector.tensor_mul(out=rot[:], in0=rot[:], in1=sin_t[:])
                        nc.vector.tensor_mul(out=dst, in0=dst, in1=cos_t[:])
                        nc.vector.tensor_add(out=dst, in0=dst, in1=rot[:])

            # ---- phase 2: attention + output projection ----
            # yproj of unit i is emitted after attention of unit i+1 (software
            # pipelining): the PE then has scores/AV matmuls to run while unit
            # i's normalization chain (recip -> broadcast -> mul) completes.
            def emit_yproj(onorm, b, qt):
                for s4 in range(4):
                    r0 = b * T + qt * 512 + s4 * P
                    ystage = ysp.tile([P, D], f32, tag="ystage")
                    for dout in range(4):
                        py = ps.tile([P, 512], f32, tag="ps", name="py")
                        for h in range(HPC):
                            nc.tensor.matmul(
                                py[:],
                                onorm[:, h, s4 * P:(s4 + 1) * P],
                                wo_t[:, h, dout * 512:(dout + 1) * 512],
                                start=(h == 0), stop=(h == HPC - 1))
                        nc.scalar.copy(ystage[:, dout * 512:(dout + 1) * 512], py[:])
                    nc.sync.dma_start(y[r0:r0 + P, :], ystage[:])

            pending = []

            def emit_attn(b, qt):
                    qsl = slice(b * T + qt * 512, b * T + qt * 512 + 512)
                    onorm = onp.tile([P, HPC, 512], f32r, tag="onorm")
                    for h in range(HPC):
                        qr = qT_t[:, h, qsl]
                        nkt = KT_PER_Q * (qt + 1)
                        po = ps.tile([P, 512], f32, tag="ps")
                        pr = ps.tile([P, 512], f32, tag="ps")

                        def emit_score(kt, b=b, qt=qt, h=h, qr=qr):
                            ksl = slice(b * T + kt * P, b * T + (kt + 1) * P)
                            pscore = ps.tile([P, 512], f32, tag="ps", name="pscore")
                            nc.tensor.matmul(pscore[:], kT_t[:, h, ksl],
                                             qr, start=True, stop=True)
                            ptile = ptp.tile([P, 512], bf16, tag="pt", name="ptile")
                            nc.scalar.activation(ptile[:], pscore[:],
                                                 mybir.ActivationFunctionType.Exp,
                                                 scale=inv_sqrt_hd)
                            if kt >= KT_PER_Q * qt:
                                nc.vector.tensor_mul(out=ptile[:], in0=ptile[:],
                                                     in1=masks[kt - KT_PER_Q * qt][:])
                            return ptile

                        # kt loop pipelined by one: scores for kt+1 are issued
                        # before the exp-gated AV/ones matmuls of kt, so the PE
                        # always has wait-free work while ACT runs exp.
                        ptiles = {0: emit_score(0)}
                        for kt in range(nkt):
                            if kt + 1 < nkt:
                                ptiles[kt + 1] = emit_score(kt + 1)
                            ptile = ptiles.pop(kt)
                            st, sp = (kt == 0), (kt == nkt - 1)
                            nc.tensor.matmul(po[:], v_t[:, b * (T // P) + kt,
                                                        h * HD:(h + 1) * HD],
                                             ptile[:], start=st, stop=sp)
                            nc.tensor.matmul(pr[0:1, :], ones_col[:], ptile[:],
                                             start=st, stop=sp)
                        # copy o out of PSUM right away (frees the bank), then
                        # normalize in place once 1/r is broadcast.
                        nc.scalar.copy(onorm[:, h, :], po[:])
                        rr = rrp.tile([1, 512], f32, tag="rr")
                        nc.vector.reciprocal(rr[:], pr[0:1, :])
                        bc = bcp.tile([P, 512], f32, tag="bc")
                        nc.gpsimd.partition_broadcast(bc[:], rr[:])
                        nc.vector.tensor_mul(out=onorm[:, h, :],
                                             in0=onorm[:, h, :], in1=bc[:])

                    pending.append((onorm, b, qt))
                    if len(pending) > 2:
                        emit_yproj(*pending.pop(0))

            # ---- schedule ----
            for tt in range(TT):
                emit_tile(tt)
                if tt == 3:
                    for h in range(HPC):
                        nc.scalar.dma_start(
                            wo_t[:, h, :],
                            woT.rearrange("(ko ki) n -> ki ko n", ki=P)[:, h, :])
            for b in range(B):
                for qt in range(QT):
                    emit_attn(b, qt)
            for p_ in pending:
                emit_yproj(*p_)

    nc.compile()
    return nc


def get_nc():
    if "nc" not in _CACHE:
        _CACHE["nc"] = _build_nc()
    return _CACHE["nc"]


def make_in_maps(x, cos, sin, wq, wk, wv, wo):
    xT = x.reshape(TOK, D).T  # [D, TOK]
    xTt = np.ascontiguousarray(
        xT.reshape(CCHUNKS, P, TT, 512).transpose(2, 0, 1, 3))
    cosT = np.ascontiguousarray(cos.reshape(TOK, HD).T)
    sinT = np.ascontiguousarray(sin.reshape(TOK, HD).T)
    in_maps = []
    for c in range(NCORES):
        dsl = slice(c * DC, (c + 1) * DC)
        in_maps.append({
            "xTt": xTt,
            "cosT": cosT,
            "sinT": sinT,
            "wqT": np.ascontiguousarray(wq[dsl, :].T),
            "wkT": np.ascontiguousarray(wk[dsl, :].T),
            "wvT": np.ascontiguousarray(wv[dsl, :].T),
            "woT": np.ascontiguousarray(wo[:, dsl].T),
        })
    return in_maps


def kernel(x, cos, sin, wq, wk, wv, wo):
    from concourse.bass_utils import run_bass_kernel_spmd

    nc = get_nc()
    in_maps = make_in_maps(
        np.asarray(x, dtype=np.float32), np.asarray(cos, dtype=np.float32),
        np.asarray(sin, dtype=np.float32), np.asarray(wq, dtype=np.float32),
        np.asarray(wk, dtype=np.float32), np.asarray(wv, dtype=np.float32),
        np.asarray(wo, dtype=np.float32))
    res = run_bass_kernel_spmd(nc, in_maps, list(range(NCORES)))
    out = np.zeros((TOK, D), dtype=np.float64)
    for m in res.results:
        out += m["y"].astype(np.float64)
    return out.astype(np.float32).reshape(B, T, D)


# revision 32
# speedup vs baseline: 1.0859x; 1.0595x over previous
"""Trainium2 Bass kernel for causal multi-head attention with RoPE.

Problem: x[2,2048,2048], 16 heads, head_dim 128, fp32.
  q/k/v = x @ w{q,k,v}^T ; RoPE on q,k ; causal softmax(q k^T / sqrt(128)) @ v ; out @ wo^T

Sharding: Megatron tensor-parallel over heads — 2 heads per core on 8 cores.
Each core computes a partial y (its 2 heads' contribution through wo); the host
sums the 8 partials.  No device collectives.

Per-core layout strategy (all matmuls fp32r at free-dim >= 256, probs bf16):
  - xT [2048, 4096]  (feature-major activations, host-pre-transposed)
  - q^T, k^T computed feature-major [head_dim, tokens]; v token-major [tokens, d]
  - scores computed transposed: S^T[key, q] = kT.T @ qT  (single K=128 pass)
  - softmax WITHOUT max subtraction (scores bounded ~ +-10, exp is safe in fp32):
      P^T = exp(S^T / sqrt(128)) (ACT engine, fused scale), causal mask by
      multiplying bf16 0/1 mask tiles, row-sum r via ones-column matmul,
      o^T = v.T @ P^T accumulated in PSUM, normalized by broadcast(1/r).
  - y rows = (o_norm^T).T @ woT, written token-major straight to DRAM.
"""

import math
import sys

sys.path.insert(0, "/opt/trn_rl_repo")

import ml_dtypes  # noqa: E402
import numpy as np  # noqa: E402

P = 128
D = 2048
HD = 128  # head dim
B = 2
T = 2048
TOK = B * T  # 4096
NCORES = 8
HPC = 2  # heads per core
DC = HPC * HD  # 256 dims per core
CCHUNKS = D // P  # 16 contraction chunks
TT = TOK // 512  # 8 token tiles of 512
QT = T // 512  # 4 query tiles per batch
KT_PER_Q = 512 // P  # 4 key tiles per query tile

_CACHE = {}


def _build_nc():
    import concourse.bacc as bacc
    import concourse.mybir as mybir
    import concourse.tile as tile

    f32 = mybir.dt.float32
    f32r = mybir.dt.float32r
    bf16 = mybir.dt.bfloat16

    nc = bacc.Bacc("TRN2", target_bir_lowering=False, debug=False, num_devices=NCORES)

    # x pre-tiled on host: [tt, c_chunk, 128, 512], each chunk contiguous
    xTt = nc.dram_tensor("xTt", [TT, CCHUNKS, P, 512], f32r,
                         kind="ExternalInput").ap()
    cosT = nc.dram_tensor("cosT", [HD, TOK], f32, kind="ExternalInput").ap()
    sinT = nc.dram_tensor("sinT", [HD, TOK], f32, kind="ExternalInput").ap()
    wqT = nc.dram_tensor("wqT", [D, DC], f32r, kind="ExternalInput").ap()
    wkT = nc.dram_tensor("wkT", [D, DC], f32r, kind="ExternalInput").ap()
    wvT = nc.dram_tensor("wvT", [D, DC], f32r, kind="ExternalInput").ap()
    woT = nc.dram_tensor("woT", [DC, D], bf16, kind="ExternalInput").ap()
    y = nc.dram_tensor("y", [TOK, D], f32, kind="ExternalOutput").ap()

    inv_sqrt_hd = 1.0 / math.sqrt(HD)

    with tile.TileContext(nc) as tc:
        with (
            tc.tile_pool(name="consts", bufs=1) as consts,
            tc.tile_pool(name="wpool", bufs=1) as wpool,
            tc.tile_pool(name="qkv", bufs=1) as qkv,
            tc.tile_pool(name="xp", bufs=4) as xp,
            tc.tile_pool(name="csp", bufs=2) as csp,
            tc.tile_pool(name="ropep", bufs=1) as ropep,
            tc.tile_pool(name="ptp", bufs=4) as ptp,
            tc.tile_pool(name="rrp", bufs=2) as rrp,
            tc.tile_pool(name="bcp", bufs=2) as bcp,
            tc.tile_pool(name="onp", bufs=3) as onp,
            tc.tile_pool(name="ysp", bufs=2) as ysp,
            tc.tile_pool(name="ps", bufs=8, space="PSUM") as ps,
        ):
            # ---- constants ----
            # causal 0/1 bf16 masks for the 4 diagonal-crossing offsets
            masks = []
            for mi in range(KT_PER_Q):
                m = consts.tile([P, 512], bf16, tag=f"mask{mi}")
                nc.gpsimd.memset(m[:], 1.0)
                # keep where (q_local - key_local) >= 0:  f - p - 128*mi >= 0
                nc.gpsimd.affine_select(
                    out=m[:], in_=m[:], compare_op=mybir.AluOpType.is_ge,
                    fill=0.0, base=-P * mi, channel_multiplier=-1, pattern=[[1, 512]],
                )
                masks.append(m)
            ones_col = consts.tile([P, 1], bf16, tag="ones_col")
            nc.gpsimd.memset(ones_col[:], 1.0)

            # ---- resident weights.  Per-c-chunk DMAs are emitted inside the
            # first token tile's c-loop so the x-tile stream is not queued
            # behind 8 MiB of weight traffic; wo loads after phase 1. ----
            wq_t = wpool.tile([P, CCHUNKS, DC], f32r, tag="wq")
            wk_t = wpool.tile([P, CCHUNKS, DC], f32r, tag="wk")
            wv_t = wpool.tile([P, CCHUNKS, DC], f32r, tag="wv")
            wo_t = wpool.tile([P, HPC, D], bf16, tag="wo")

            def emit_w_chunk(c):
                for wt, wdram in ((wq_t, wqT), (wk_t, wkT), (wv_t, wvT)):
                    nc.sync.dma_start(
                        wt[:, c:c + 1, :],
                        wdram.rearrange("(co ci) d -> ci co d", ci=P)[:, c:c + 1, :])

            # ---- resident activations ----
            qT_t = qkv.tile([P, HPC, TOK], f32r, tag="qT")  # [head_dim, h, tok]
            kT_t = qkv.tile([P, HPC, TOK], f32r, tag="kT")
            v_t = qkv.tile([P, TOK // P, DC], bf16, tag="v")  # [tok%128, tokblk, d]

            # ---- phase 1 tile body ----
            def emit_tile(tt):
                tsl = slice(tt * 512, (tt + 1) * 512)
                cos_t = csp.tile([P, 512], f32, tag="cos")
                nc.scalar.dma_start(cos_t[:], cosT[:, tsl])
                sin_t = csp.tile([P, 512], f32, tag="sin")
                nc.scalar.dma_start(sin_t[:], sinT[:, tsl])

                pq = [ps.tile([P, 512], f32, tag="ps", name=f"pq{i}") for i in range(HPC)]
                pk = [ps.tile([P, 512], f32, tag="ps", name=f"pk{i}") for i in range(HPC)]
                # two banks hold all four v accumulators ([t128, 256] pairs packed
                # side by side).  Only the first half's c==0 matmul uses start=True
                # (clears the whole bank); the second half's first matmul then
                # overwrites its still-clean elements via has_written bits.
                pv = [ps.tile([P, 512], f32, tag="ps", name=f"pv{i}") for i in range(2)]

                for c in range(CCHUNKS):
                    if tt == 0 and c == 0:
                        for cc in range(3):
                            emit_w_chunk(cc)
                    if tt == 0 and c + 3 < CCHUNKS:
                        emit_w_chunk(c + 3)
                    xt = xp.tile([P, 512], f32r, tag="x")
                    nc.sync.dma_start(xt[:], xTt[tt, c])
                    xtr = xt[:]
                    st, sp = (c == 0), (c == CCHUNKS - 1)
                    for h in range(HPC):
                        dsl = slice(h * HD, (h + 1) * HD)
                        nc.tensor.matmul(pq[h][:], wq_t[:, c, dsl], xtr,
                                         start=st, stop=sp)
                        nc.tensor.matmul(pk[h][:], wk_t[:, c, dsl], xtr,
                                         start=st, stop=sp)
                    for s4 in range(4):
                        half = s4 % 2
                        nc.tensor.matmul(pv[s4 // 2][:, half * DC:(half + 1) * DC],
                                         xt[:, s4 * P:(s4 + 1) * P],
                                         wv_t[:, c, :],
                                         start=st and half == 0, stop=sp,
                                         skip_group_check=half == 1)

                # Free all six PSUM banks as fast as possible: raw q + v copies
                # on ACT, raw k copies on DVE (parallel engines), then run RoPE
                # in place from SBUF.
                for h in range(HPC):
                    nc.scalar.copy(qT_t[:, h, tsl], pq[h][:])
                for h in range(HPC):
                    nc.vector.tensor_copy(kT_t[:, h, tsl], pk[h][:])
                for s4 in range(4):
                    half = s4 % 2
                    nc.scalar.copy(v_t[:, tt * 4 + s4, :],
                                   pv[s4 // 2][:, half * DC:(half + 1) * DC])
                # RoPE: dst = raw*cos + rot(raw)*sin (rot: [0:64]=-raw[64:], [64:]=raw[:64])
                for dst_t in (qT_t, kT_t):
                    for h in range(HPC):
                        dst = dst_t[:, h, tsl]
                        rot = ropep.tile([P, 512], f32, tag="rot")
                        nc.scalar.mul(rot[0:64, :], dst[64:128, :], -1.0)
                        nc.scalar.copy(rot[64:128, :], dst[0:64, :])
                        nc.vector.tensor_mul(out=rot[:], in0=rot[:], in1=sin_t[:])
                        nc.vector.tensor_mul(out=dst, in0=dst, in1=cos_t[:])
                        nc.vector.tensor_add(out=dst, in0=dst, in1=rot[:])

            # ---- phase 2: attention + output projection ----
            # yproj of unit i is emitted after attention of unit i+1 (software
            # pipelining): the PE then has scores/AV matmuls to run while unit
            # i's normalization chain (recip -> broadcast -> mul) completes.
            def emit_yproj(onorm, b, qt):
                for s4 in range(4):
                    r0 = b * T + qt * 512 + s4 * P
                    ystage = ysp.tile([P, D], f32, tag="ystage")
                    for dout in range(4):
                        py = ps.tile([P, 512], f32, tag="ps", name="py")
                        for h in range(HPC):
                            nc.tensor.matmul(
                                py[:],
                                onorm[:, h, s4 * P:(s4 + 1) * P],
                                wo_t[:, h, dout * 512:(dout + 1) * 512],
                                start=(h == 0), stop=(h == HPC - 1))
                        nc.scalar.copy(ystage[:, dout * 512:(dout + 1) * 512], py[:])
                    nc.sync.dma_start(y[r0:r0 + P, :], ystage[:])

            pending = []

            def emit_attn(b, qt):
                    qsl = slice(b * T + qt * 512, b * T + qt * 512 + 512)
                    onorm = onp.tile([P, HPC, 512], bf16, tag="onorm")
                    for h in range(HPC):
                        qr = qT_t[:, h, qsl]
                        nkt = KT_PER_Q * (qt + 1)
                        po = ps.tile([P, 512], f32, tag="ps")
                        pr = ps.tile([P, 512], f32, tag="ps")

                        def emit_score(kt, b=b, qt=qt, h=h, qr=qr):
                            ksl = slice(b * T + kt * P, b * T + (kt + 1) * P)
                            pscore = ps.tile([P, 512], f32, tag="ps", name="pscore")
                            nc.tensor.matmul(pscore[:], kT_t[:, h, ksl],
                                             qr, start=True, stop=True)
                            ptile = ptp.tile([P, 512], bf16, tag="pt", name="ptile")
                            nc.scalar.activation(ptile[:], pscore[:],
                                                 mybir.ActivationFunctionType.Exp,
                                                 scale=inv_sqrt_hd)
                            if kt >= KT_PER_Q * qt:
                                nc.vector.tensor_mul(out=ptile[:], in0=ptile[:],
                                                     in1=masks[kt - KT_PER_Q * qt][:])
                            return ptile

                        # kt loop pipelined by one: scores for kt+1 are issued
                        # before the exp-gated AV/ones matmuls of kt, so the PE
                        # always has wait-free work while ACT runs exp.
                        ptiles = {0: emit_score(0)}
                        for kt in range(nkt):
                            if kt + 1 < nkt:
                                ptiles[kt + 1] = emit_score(kt + 1)
                            ptile = ptiles.pop(kt)
                            st, sp = (kt == 0), (kt == nkt - 1)
                            nc.tensor.matmul(po[:], v_t[:, b * (T // P) + kt,
                                                        h * HD:(h + 1) * HD],
                                             ptile[:], start=st, stop=sp)
                            nc.tensor.matmul(pr[0:1, :], ones_col[:], ptile[:],
                                             start=st, stop=sp)
                        # copy o out of PSUM right away (frees the bank), then
                        # normalize in place once 1/r is broadcast.
                        nc.scalar.copy(onorm[:, h, :], po[:])
                        rr = rrp.tile([1, 512], f32, tag="rr")
                        nc.vector.reciprocal(rr[:], pr[0:1, :])
                        bc = bcp.tile([P, 512], f32, tag="bc")
                        nc.gpsimd.partition_broadcast(bc[:], rr[:])
                        nc.vector.tensor_mul(out=onorm[:, h, :],
                                             in0=onorm[:, h, :], in1=bc[:])

                    pending.append((onorm, b, qt))
                    if len(pending) > 2:
                        emit_yproj(*pending.pop(0))

            # ---- schedule ----
            for tt in range(TT):
                emit_tile(tt)
                if tt == 3:
                    for h in range(HPC):
                        nc.scalar.dma_start(
                            wo_t[:, h, :],
                            woT.rearrange("(ko ki) n -> ki ko n", ki=P)[:, h, :])
            for b in range(B):
                for qt in range(QT):
                    emit_attn(b, qt)
            for p_ in pending:
                emit_yproj(*p_)

    nc.compile()
    return nc


def get_nc():
    if "nc" not in _CACHE:
        _CACHE["nc"] = _build_nc()
    return _CACHE["nc"]


def make_in_maps(x, cos, sin, wq, wk, wv, wo):
    xT = x.reshape(TOK, D).T  # [D, TOK]
    xTt = np.ascontiguousarray(
        xT.reshape(CCHUNKS, P, TT, 512).transpose(2, 0, 1, 3))
    cosT = np.ascontiguousarray(cos.reshape(TOK, HD).T)
    sinT = np.ascontiguousarray(sin.reshape(TOK, HD).T)
    in_maps = []
    for c in range(NCORES):
        dsl = slice(c * DC, (c + 1) * DC)
        in_maps.append({
            "xTt": xTt,
            "cosT": cosT,
            "sinT": sinT,
            "wqT": np.ascontiguousarray(wq[dsl, :].T),
            "wkT": np.ascontiguousarray(wk[dsl, :].T),
            "wvT": np.ascontiguousarray(wv[dsl, :].T),
            "woT": np.ascontiguousarray(wo[:, dsl].T).astype(ml_dtypes.bfloat16),
        })
    return in_maps


def kernel(x, cos, sin, wq, wk, wv, wo):
    from concourse.bass_utils import run_bass_kernel_spmd

    nc = get_nc()
    in_maps = make_in_maps(
        np.asarray(x, dtype=np.float32), np.asarray(cos, dtype=np.float32),
        np.asarray(sin, dtype=np.float32), np.asarray(wq, dtype=np.float32),
        np.asarray(wk, dtype=np.float32), np.asarray(wv, dtype=np.float32),
        np.asarray(wo, dtype=np.float32))
    res = run_bass_kernel_spmd(nc, in_maps, list(range(NCORES)))
    out = np.zeros((TOK, D), dtype=np.float64)
    for m in res.results:
        out += m["y"].astype(np.float64)
    return out.astype(np.float32).reshape(B, T, D)


# revision 33
# speedup vs baseline: 1.0932x; 1.0067x over previous
"""Trainium2 Bass kernel for causal multi-head attention with RoPE.

Problem: x[2,2048,2048], 16 heads, head_dim 128, fp32.
  q/k/v = x @ w{q,k,v}^T ; RoPE on q,k ; causal softmax(q k^T / sqrt(128)) @ v ; out @ wo^T

Sharding: Megatron tensor-parallel over heads — 2 heads per core on 8 cores.
Each core computes a partial y (its 2 heads' contribution through wo); the host
sums the 8 partials.  No device collectives.

Per-core layout strategy (all matmuls fp32r at free-dim >= 256, probs bf16):
  - xT [2048, 4096]  (feature-major activations, host-pre-transposed)
  - q^T, k^T computed feature-major [head_dim, tokens]; v token-major [tokens, d]
  - scores computed transposed: S^T[key, q] = kT.T @ qT  (single K=128 pass)
  - softmax WITHOUT max subtraction (scores bounded ~ +-10, exp is safe in fp32):
      P^T = exp(S^T / sqrt(128)) (ACT engine, fused scale), causal mask by
      multiplying bf16 0/1 mask tiles, row-sum r via ones-column matmul,
      o^T = v.T @ P^T accumulated in PSUM, normalized by broadcast(1/r).
  - y rows = (o_norm^T).T @ woT, written token-major straight to DRAM.
"""

import math
import sys

sys.path.insert(0, "/opt/trn_rl_repo")

import ml_dtypes  # noqa: E402
import numpy as np  # noqa: E402

P = 128
D = 2048
HD = 128  # head dim
B = 2
T = 2048
TOK = B * T  # 4096
NCORES = 8
HPC = 2  # heads per core
DC = HPC * HD  # 256 dims per core
CCHUNKS = D // P  # 16 contraction chunks
TT = TOK // 512  # 8 token tiles of 512
QT = T // 512  # 4 query tiles per batch
KT_PER_Q = 512 // P  # 4 key tiles per query tile

_CACHE = {}


def _build_nc():
    import concourse.bacc as bacc
    import concourse.mybir as mybir
    import concourse.tile as tile

    f32 = mybir.dt.float32
    f32r = mybir.dt.float32r
    bf16 = mybir.dt.bfloat16

    nc = bacc.Bacc("TRN2", target_bir_lowering=False, debug=False, num_devices=NCORES)

    # x pre-tiled on host: [tt, c_chunk, 128, 512], each chunk contiguous
    xTt = nc.dram_tensor("xTt", [TT, CCHUNKS, P, 512], f32r,
                         kind="ExternalInput").ap()
    cosT = nc.dram_tensor("cosT", [HD, TOK], f32, kind="ExternalInput").ap()
    sinT = nc.dram_tensor("sinT", [HD, TOK], f32, kind="ExternalInput").ap()
    wqT = nc.dram_tensor("wqT", [D, DC], f32r, kind="ExternalInput").ap()
    wkT = nc.dram_tensor("wkT", [D, DC], f32r, kind="ExternalInput").ap()
    wvT = nc.dram_tensor("wvT", [D, DC], f32r, kind="ExternalInput").ap()
    woT = nc.dram_tensor("woT", [DC, D], bf16, kind="ExternalInput").ap()
    y = nc.dram_tensor("y", [TOK, D], f32, kind="ExternalOutput").ap()

    inv_sqrt_hd = 1.0 / math.sqrt(HD)

    with tile.TileContext(nc) as tc:
        with (
            tc.tile_pool(name="consts", bufs=1) as consts,
            tc.tile_pool(name="wpool", bufs=1) as wpool,
            tc.tile_pool(name="qkv", bufs=1) as qkv,
            tc.tile_pool(name="xp", bufs=6) as xp,
            tc.tile_pool(name="csp", bufs=2) as csp,
            tc.tile_pool(name="ropep", bufs=2) as ropep,
            tc.tile_pool(name="ptp", bufs=6) as ptp,
            tc.tile_pool(name="rrp", bufs=2) as rrp,
            tc.tile_pool(name="bcp", bufs=2) as bcp,
            tc.tile_pool(name="onp", bufs=3) as onp,
            tc.tile_pool(name="ysp", bufs=2) as ysp,
            tc.tile_pool(name="ps", bufs=8, space="PSUM") as ps,
        ):
            # ---- constants ----
            # causal 0/1 bf16 masks for the 4 diagonal-crossing offsets
            masks = []
            for mi in range(KT_PER_Q):
                m = consts.tile([P, 512], bf16, tag=f"mask{mi}")
                nc.gpsimd.memset(m[:], 1.0)
                # keep where (q_local - key_local) >= 0:  f - p - 128*mi >= 0
                nc.gpsimd.affine_select(
                    out=m[:], in_=m[:], compare_op=mybir.AluOpType.is_ge,
                    fill=0.0, base=-P * mi, channel_multiplier=-1, pattern=[[1, 512]],
                )
                masks.append(m)
            ones_col = consts.tile([P, 1], bf16, tag="ones_col")
            nc.gpsimd.memset(ones_col[:], 1.0)

            # ---- resident weights.  Per-c-chunk DMAs are emitted inside the
            # first token tile's c-loop so the x-tile stream is not queued
            # behind 8 MiB of weight traffic; wo loads after phase 1. ----
            wq_t = wpool.tile([P, CCHUNKS, DC], f32r, tag="wq")
            wk_t = wpool.tile([P, CCHUNKS, DC], f32r, tag="wk")
            wv_t = wpool.tile([P, CCHUNKS, DC], f32r, tag="wv")
            wo_t = wpool.tile([P, HPC, D], bf16, tag="wo")

            def emit_w_chunk(c):
                for wt, wdram in ((wq_t, wqT), (wk_t, wkT), (wv_t, wvT)):
                    nc.sync.dma_start(
                        wt[:, c:c + 1, :],
                        wdram.rearrange("(co ci) d -> ci co d", ci=P)[:, c:c + 1, :])

            # ---- resident activations ----
            qT_t = qkv.tile([P, HPC, TOK], f32r, tag="qT")  # [head_dim, h, tok]
            kT_t = qkv.tile([P, HPC, TOK], f32r, tag="kT")
            v_t = qkv.tile([P, TOK // P, DC], bf16, tag="v")  # [tok%128, tokblk, d]

            # ---- phase 1 tile body ----
            def emit_tile(tt):
                tsl = slice(tt * 512, (tt + 1) * 512)
                cos_t = csp.tile([P, 512], f32, tag="cos")
                nc.scalar.dma_start(cos_t[:], cosT[:, tsl])
                sin_t = csp.tile([P, 512], f32, tag="sin")
                nc.scalar.dma_start(sin_t[:], sinT[:, tsl])

                pq = [ps.tile([P, 512], f32, tag="ps", name=f"pq{i}") for i in range(HPC)]
                pk = [ps.tile([P, 512], f32, tag="ps", name=f"pk{i}") for i in range(HPC)]
                # two banks hold all four v accumulators ([t128, 256] pairs packed
                # side by side).  Only the first half's c==0 matmul uses start=True
                # (clears the whole bank); the second half's first matmul then
                # overwrites its still-clean elements via has_written bits.
                pv = [ps.tile([P, 512], f32, tag="ps", name=f"pv{i}") for i in range(2)]

                for c in range(CCHUNKS):
                    if tt == 0 and c == 0:
                        for cc in range(3):
                            emit_w_chunk(cc)
                    if tt == 0 and c + 3 < CCHUNKS:
                        emit_w_chunk(c + 3)
                    xt = xp.tile([P, 512], f32r, tag="x")
                    nc.sync.dma_start(xt[:], xTt[tt, c])
                    xtr = xt[:]
                    st, sp = (c == 0), (c == CCHUNKS - 1)
                    for h in range(HPC):
                        dsl = slice(h * HD, (h + 1) * HD)
                        nc.tensor.matmul(pq[h][:], wq_t[:, c, dsl], xtr,
                                         start=st, stop=sp)
                        nc.tensor.matmul(pk[h][:], wk_t[:, c, dsl], xtr,
                                         start=st, stop=sp)
                    for s4 in range(4):
                        half = s4 % 2
                        nc.tensor.matmul(pv[s4 // 2][:, half * DC:(half + 1) * DC],
                                         xt[:, s4 * P:(s4 + 1) * P],
                                         wv_t[:, c, :],
                                         start=st and half == 0, stop=sp,
                                         skip_group_check=half == 1)

                # Free all six PSUM banks as fast as possible: raw q + v copies
                # on ACT, raw k copies on DVE (parallel engines), then run RoPE
                # in place from SBUF.
                for h in range(HPC):
                    nc.scalar.copy(qT_t[:, h, tsl], pq[h][:])
                for h in range(HPC):
                    nc.vector.tensor_copy(kT_t[:, h, tsl], pk[h][:])
                for s4 in range(4):
                    half = s4 % 2
                    nc.scalar.copy(v_t[:, tt * 4 + s4, :],
                                   pv[s4 // 2][:, half * DC:(half + 1) * DC])
                # RoPE: dst = raw*cos + rot(raw)*sin (rot: [0:64]=-raw[64:], [64:]=raw[:64])
                for dst_t in (qT_t, kT_t):
                    for h in range(HPC):
                        dst = dst_t[:, h, tsl]
                        rot = ropep.tile([P, 512], f32, tag="rot")
                        nc.scalar.mul(rot[0:64, :], dst[64:128, :], -1.0)
                        nc.scalar.copy(rot[64:128, :], dst[0:64, :])
                        nc.vector.tensor_mul(out=rot[:], in0=rot[:], in1=sin_t[:])
                        nc.vector.tensor_mul(out=dst, in0=dst, in1=cos_t[:])
                        nc.vector.tensor_add(out=dst, in0=dst, in1=rot[:])

            # ---- phase 2: attention + output projection ----
            # yproj of unit i is emitted after attention of unit i+1 (software
            # pipelining): the PE then has scores/AV matmuls to run while unit
            # i's normalization chain (recip -> broadcast -> mul) completes.
            def emit_yproj(onorm, b, qt):
                for s4 in range(4):
                    r0 = b * T + qt * 512 + s4 * P
                    ystage = ysp.tile([P, D], f32, tag="ystage")
                    for dout in range(4):
                        py = ps.tile([P, 512], f32, tag="ps", name="py")
                        for h in range(HPC):
                            nc.tensor.matmul(
                                py[:],
                                onorm[:, h, s4 * P:(s4 + 1) * P],
                                wo_t[:, h, dout * 512:(dout + 1) * 512],
                                start=(h == 0), stop=(h == HPC - 1))
                        nc.scalar.copy(ystage[:, dout * 512:(dout + 1) * 512], py[:])
                    nc.sync.dma_start(y[r0:r0 + P, :], ystage[:])

            pending = []

            def emit_attn(b, qt):
                    qsl = slice(b * T + qt * 512, b * T + qt * 512 + 512)
                    onorm = onp.tile([P, HPC, 512], bf16, tag="onorm")
                    for h in range(HPC):
                        qr = qT_t[:, h, qsl]
                        nkt = KT_PER_Q * (qt + 1)
                        po = ps.tile([P, 512], f32, tag="ps")
                        pr = ps.tile([P, 512], f32, tag="ps")

                        def emit_score(kt, b=b, qt=qt, h=h, qr=qr):
                            ksl = slice(b * T + kt * P, b * T + (kt + 1) * P)
                            pscore = ps.tile([P, 512], f32, tag="ps", name="pscore")
                            nc.tensor.matmul(pscore[:], kT_t[:, h, ksl],
                                             qr, start=True, stop=True)
                            ptile = ptp.tile([P, 512], bf16, tag="pt", name="ptile")
                            nc.scalar.activation(ptile[:], pscore[:],
                                                 mybir.ActivationFunctionType.Exp,
                                                 scale=inv_sqrt_hd)
                            if kt >= KT_PER_Q * qt:
                                nc.vector.tensor_mul(out=ptile[:], in0=ptile[:],
                                                     in1=masks[kt - KT_PER_Q * qt][:])
                            return ptile

                        # kt loop pipelined by one: scores for kt+1 are issued
                        # before the exp-gated AV/ones matmuls of kt, so the PE
                        # always has wait-free work while ACT runs exp.
                        ptiles = {0: emit_score(0)}
                        for kt in range(nkt):
                            if kt + 1 < nkt:
                                ptiles[kt + 1] = emit_score(kt + 1)
                            ptile = ptiles.pop(kt)
                            st, sp = (kt == 0), (kt == nkt - 1)
                            nc.tensor.matmul(po[:], v_t[:, b * (T // P) + kt,
                                                        h * HD:(h + 1) * HD],
                                             ptile[:], start=st, stop=sp)
                            nc.tensor.matmul(pr[0:1, :], ones_col[:], ptile[:],
                                             start=st, stop=sp)
                        # copy o out of PSUM right away (frees the bank), then
                        # normalize in place once 1/r is broadcast.
                        nc.scalar.copy(onorm[:, h, :], po[:])
                        rr = rrp.tile([1, 512], f32, tag="rr")
                        nc.vector.reciprocal(rr[:], pr[0:1, :])
                        bc = bcp.tile([P, 512], f32, tag="bc")
                        nc.gpsimd.partition_broadcast(bc[:], rr[:])
                        nc.vector.tensor_mul(out=onorm[:, h, :],
                                             in0=onorm[:, h, :], in1=bc[:])

                    pending.append((onorm, b, qt))
                    if len(pending) > 2:
                        emit_yproj(*pending.pop(0))

            # ---- schedule ----
            for tt in range(TT):
                emit_tile(tt)
                if tt == 3:
                    for h in range(HPC):
                        nc.scalar.dma_start(
                            wo_t[:, h, :],
                            woT.rearrange("(ko ki) n -> ki ko n", ki=P)[:, h, :])
            for b in range(B):
                for qt in range(QT):
                    emit_attn(b, qt)
            for p_ in pending:
                emit_yproj(*p_)

    nc.compile()
    return nc


def get_nc():
    if "nc" not in _CACHE:
        _CACHE["nc"] = _build_nc()
    return _CACHE["nc"]


def make_in_maps(x, cos, sin, wq, wk, wv, wo):
    xT = x.reshape(TOK, D).T  # [D, TOK]
    xTt = np.ascontiguousarray(
        xT.reshape(CCHUNKS, P, TT, 512).transpose(2, 0, 1, 3))
    cosT = np.ascontiguousarray(cos.reshape(TOK, HD).T)
    sinT = np.ascontiguousarray(sin.reshape(TOK, HD).T)
    in_maps = []
    for c in range(NCORES):
        dsl = slice(c * DC, (c + 1) * DC)
        in_maps.append({
            "xTt": xTt,
            "cosT": cosT,
            "sinT": sinT,
            "wqT": np.ascontiguousarray(wq[dsl, :].T),
            "wkT": np.ascontiguousarray(wk[dsl, :].T),
            "wvT": np.ascontiguousarray(wv[dsl, :].T),
            "woT": np.ascontiguousarray(wo[:, dsl].T).astype(ml_dtypes.bfloat16),
        })
    return in_maps


def kernel(x, cos, sin, wq, wk, wv, wo):
    from concourse.bass_utils import run_bass_kernel_spmd

    nc = get_nc()
    in_maps = make_in_maps(
        np.asarray(x, dtype=np.float32), np.asarray(cos, dtype=np.float32),
        np.asarray(sin, dtype=np.float32), np.asarray(wq, dtype=np.float32),
        np.asarray(wk, dtype=np.float32), np.asarray(wv, dtype=np.float32),
        np.asarray(wo, dtype=np.float32))
    res = run_bass_kernel_spmd(nc, in_maps, list(range(NCORES)))
    out = np.zeros((TOK, D), dtype=np.float64)
    for m in res.results:
        out += m["y"].astype(np.float64)
    return out.astype(np.float32).reshape(B, T, D)
